# revision 22
# baseline (speedup 1.0000x reference)
"""Trainium2 Bass kernel for DirectVolumeRenderer (nn_DirectVolumeRenderer).

Strategy
--------
The camera in this problem is axis-aligned (R=I), so for every depth step p
all 128x128 ray sample points lie on an axis-aligned uniform grid: z is
constant, x depends only on the pixel column, y only on the pixel row.
Trilinear sampling of a depth slice therefore factorizes into dense matmuls

    S_p = Wy_p^T @ Vlerp_p @ Wx_p          (128x128 each)

where Vlerp_p = (1-wz) V[z0] + wz V[z0+1] is pre-lerped on the host (cheap)
and the matmuls run on the TensorEngine in fp16.  Only ~192 of the 256 depth
steps intersect the volume; those active slices are sharded contiguously
across the 8 cores.  Each core ray-marches its own depth segment
(emission-absorption is an associative scan), returning partial emission and
segment transmittance; the host combines out = sum_k acc_k * prod_{j<k} pk_j.
Only the feature (image3d) and density (opacity*0.1) volumes matter: the 3
RGB channels are identical copies, and the alpha channel is dropped by the
output transpose/mean.

Device pipeline (per core), slices in rounds of B=6, emission one round
delayed so the DVE stream stays dense:
  PE:   At[X, .] = Vlerp^T @ Wyt  (2 slices per PSUM-bank tile)
  ACT:  Ats(sbuf,fp16) <- At(psum,f32)      (one copy per 2 slices)
  PE:   Pcat[:, s*256:+256] = Wx^T @ Ats     (feat | dens)
  ACT:  tau[i, h, j] = 1 - Pcat_dens        (per pair; 7-col/h scan layout)
  DVE:  Cum = tensor_tensor_scan(mult, add)(tau, reset)   [A_j per (i,h)]
  GPS:  Dt = A_j - A_{j+1}                  (exact EA weights)
  DVE:  Mt = Pcat_feat * Dt ; E = reduce_add_j(Mt)
  GPS:  tmp = carry * E ; acc += tmp ; carry *= A_B
"""

import os
import sys

for _p in ("/root/.axon_site", "/root/.axon_site/_ro/trn_rl_repo",
           "/root/.axon_site/_ro/pypackages", "/opt/trn_rl_repo"):
    if os.path.isdir(_p) and _p not in sys.path:
        sys.path.append(_p)

from contextlib import ExitStack

import numpy as np

IMG_W = IMG_H = 128
N_PTS = 256
MIN_D, MAX_D = 2.0, 6.0
FOCAL = 1.7320508
SCALING = 0.1
D = H = W = 128
N_CORES = 8
B = 6                     # slices per round (Pcat = B*256 f32 = 3 PSUM banks)
BLOB_COLS = B * 512       # per-round blob: B//2 pair blocks of 1024 cols


def _pair_offsets(s):
    """Column offsets in the pair block for slice s: (vc, wy, wx)."""
    p, k = divmod(s, 2)
    base = p * 1024
    return base + k * 256, base + 512 + k * 128, base + 768 + k * 128


# ----------------------------------------------------------------- geometry

def _axis_weight_matrix(u):
    """u: [128] float voxel coords for the 128 pixels along one axis ->
    dense [128 voxel, 128 pixel] linear-interp matrix (zero outside)."""
    M = np.zeros((128, 128), np.float64)
    x0 = np.floor(u).astype(np.int64)
    frac = u - x0
    pix = np.arange(128)
    for tap, wt in ((x0, 1.0 - frac), (x0 + 1, frac)):
        valid = (tap >= 0) & (tap <= 127)
        np.add.at(M, (tap[valid], pix[valid]), wt[valid])
    return M


def _geometry(R, T):
    """Per-depth-slice separable sampling geometry (host, float64)."""
    R0 = np.asarray(R, np.float64).reshape(3, 3)
    T0 = np.asarray(T, np.float64).reshape(3)
    origin = -R0 @ T0  # origins[j] = sum_i (-T_i) R[j,i]
    xs = np.linspace(1.0, -1.0, IMG_W)
    ys = np.linspace(1.0, -1.0, IMG_H)
    dirs_cam = np.stack(np.broadcast_arrays(
        xs[None, :] / FOCAL, ys[:, None] / FOCAL, np.ones((IMG_H, IMG_W))), -1)
    dirs_world = np.einsum("hwi,ji->hwj", dirs_cam, R0)
    # separability requirement (holds for the axis-aligned camera used here)
    assert np.abs(dirs_world[:, :, 0] - dirs_world[0:1, :, 0]).max() < 1e-5
    assert np.abs(dirs_world[:, :, 1] - dirs_world[:, 0:1, 1]).max() < 1e-5
    assert np.abs(dirs_world[:, :, 2] - dirs_world[0, 0, 2]).max() < 1e-5
    d_x = dirs_world[0, :, 0]
    d_y = dirs_world[:, 0, 1]
    d_z = dirs_world[0, 0, 2]
    he = (3.0 / 128) * 127 / 2.0
    t = np.linspace(MIN_D, MAX_D, N_PTS)

    slices = []
    for p in range(N_PTS):
        ux = ((origin[0] + t[p] * d_x) / he + 1.0) * 0.5 * (W - 1)
        vy = ((origin[1] + t[p] * d_y) / he + 1.0) * 0.5 * (H - 1)
        wz = ((origin[2] + t[p] * d_z) / he + 1.0) * 0.5 * (D - 1)
        z0 = int(np.floor(wz))
        fz = wz - z0
        w0 = (1.0 - fz) if 0 <= z0 <= 127 else 0.0
        w1 = fz if 0 <= z0 + 1 <= 127 else 0.0
        if w0 == 0.0 and w1 == 0.0:
            slices.append(None)
            continue
        slices.append(dict(z0=min(max(z0, 0), 127), z1=min(max(z0 + 1, 0), 127),
                           w0=w0, w1=w1, ux=ux, vy=vy))
    return slices


# ------------------------------------------------------------- bass program

_BUILD_CACHE = {}


def _build_nc(n_slices):
    key = n_slices
    if key in _BUILD_CACHE:
        return _BUILD_CACHE[key]
    import concourse.bacc as bacc
    import concourse.mybir as mybir
    import concourse.tile as tile
    from concourse.tile import add_dep_helper

    f16 = mybir.dt.float16
    f32 = mybir.dt.float32
    mult = mybir.AluOpType.mult
    add = mybir.AluOpType.add
    sub = mybir.AluOpType.subtract
    Ident = mybir.ActivationFunctionType.Identity
    X = mybir.AxisListType.X

    n_rounds = n_slices // B
    assert n_slices % B == 0 and n_rounds >= 3

    nc = bacc.Bacc("TRN2", target_bir_lowering=False, debug=False)
    blob = nc.dram_tensor("blob", [n_rounds * (B // 2), 128, 1024], f16,
                          kind="ExternalInput")
    outs_d = nc.dram_tensor("outs", [128, 512], f32, kind="ExternalOutput")
    cum_d = nc.dram_tensor("cum_last", [128, B * 128 + 128], f32,
                           kind="ExternalOutput")
    pfs_d = nc.dram_tensor("pfs_last", [128, B * 128], f32,
                           kind="ExternalOutput")

    with tile.TileContext(nc) as tc, ExitStack() as ctx:
        pin = ctx.enter_context(tc.tile_pool(name="pin", bufs=6))
        pat = ctx.enter_context(tc.tile_pool(name="pat", bufs=2, space="PSUM"))
        ppf = ctx.enter_context(tc.tile_pool(name="ppf", bufs=2, space="PSUM"))
        ppd = ctx.enter_context(tc.tile_pool(name="ppd", bufs=1, space="PSUM"))
        pfs_p = ctx.enter_context(tc.tile_pool(name="pfs_p", bufs=2))
        pats = ctx.enter_context(tc.tile_pool(name="pats", bufs=3))
        pdt = ctx.enter_context(tc.tile_pool(name="pdt", bufs=2))
        pmt = ctx.enter_context(tc.tile_pool(name="pmt", bufs=2))
        psm = ctx.enter_context(tc.tile_pool(name="psm", bufs=2))
        pper = ctx.enter_context(tc.tile_pool(name="pper", bufs=1))

        outs = pper.tile([128, 512], f32, tag="outs")
        acc, carry = outs[:, 0:128], outs[:, 128:256]
        e2, a2 = outs[:, 256:384], outs[:, 384:512]
        rcon = pper.tile([128, B * 128 + 128], f32, tag="rcon")
        tau0 = pper.tile([128, B * 128 + 128], f32, tag="tau0")
        tau1 = pper.tile([128, B * 128 + 128], f32, tag="tau1")
        cum0 = pper.tile([128, B * 128 + 128], f32, tag="cum0")
        cum1 = pper.tile([128, B * 128 + 128], f32, tag="cum1")
        taus = [tau0, tau1]
        cums = [cum0, cum1]

        nc.gpsimd.memset(rcon[:], 0.0)
        nc.gpsimd.memset(
            rcon[:].rearrange("p (h c) -> p h c", h=128)[:, :, 0:1], 1.0)
        nc.gpsimd.memset(
            tau0[:].rearrange("p (h c) -> p h c", h=128)[:, :, 0:1], 0.0)
        nc.gpsimd.memset(
            tau1[:].rearrange("p (h c) -> p h c", h=128)[:, :, 0:1], 0.0)

        pcf_tiles = []
        scan_insts = []
        tau_insts = []
        a_tiles = {}

        def emission(q):
            """EA emission/carry ops for round q (runs one round delayed)."""
            cum3 = cums[q % 2][:].rearrange("p (h c) -> p h c", h=128)
            pf3 = (pcf_tiles[q][:].rearrange("p (j h) -> p j h", j=B)
                   .rearrange("p j h -> p h j"))
            dt = pdt.tile([128, B * 128], f32, tag="dt", name=f"dt{q}")
            dt3 = dt[:].rearrange("p (h j) -> p h j", j=B)
            nc.gpsimd.tensor_tensor(dt3, cum3[:, :, 0:B], cum3[:, :, 1:B + 1], sub)
            mt = pmt.tile([128, B * 128], f32, tag="mt", name=f"mt{q}")
            mt3 = mt[:].rearrange("p (h j) -> p h j", j=B)
            m_inst = nc.vector.tensor_tensor(mt3, pf3, dt3, mult)
            if q + 1 < len(scan_insts) and q < n_rounds - 2:
                # keep the DVE stream dense: next round's scan must issue
                # before this round's (GPSIMD-gated) multiply
                add_dep_helper(m_inst.ins, scan_insts[q + 1].ins,
                               reason="pipeline: M(q) after scan(q+1)")
            if q == 0:
                nc.vector.tensor_reduce(acc, mt3, X, add)
            elif q == n_rounds - 2:
                # host applies this round: ship E (a2 snapshotted at scan time)
                nc.vector.tensor_reduce(e2, mt3, X, add)
            else:
                e_t = psm.tile([128, 128], f32, tag="e", name=f"e{q}")
                nc.vector.tensor_reduce(e_t[:], mt3, X, add)
                tmp = psm.tile([128, 128], f32, tag="tmp", name=f"tmp{q}")
                nc.gpsimd.tensor_tensor(tmp[:], carry, e_t[:], mult)
                nc.gpsimd.tensor_tensor(acc, tmp[:], acc, add)
                nc.gpsimd.tensor_tensor(carry, carry, a_tiles[q][:], mult)

        for r in range(n_rounds):
            tau = taus[r % 2]
            tau3 = tau[:].rearrange("p (h c) -> p h c", h=128)
            cum = cums[r % 2]

            bts = []
            for p in range(B // 2):
                bt = pin.tile([128, 1024], f16, tag="blob", name=f"bt{r}_{p}")
                nc.sync.dma_start(bt[:], blob.ap()[r * (B // 2) + p])
                bts.append(bt)

            pcf = ppf.tile([128, B * 128], f32, tag="pcf", name=f"pcf{r}")
            pcf_tiles.append(pcf)
            pcd = ppd.tile([128, B * 128], f32, tag="pcd", name=f"pcd{r}")
            for p in range(B // 2):
                bt = bts[p]
                at = pat.tile([128, 512], f32, tag="at", name=f"at{r}_{p}")
                for k in range(2):
                    s = p * 2 + k
                    vo, wyo, wxo = _pair_offsets(s)
                    vob, wyob = vo - p * 1024, wyo - p * 1024
                    nc.tensor.matmul(at[:, k * 256:k * 256 + 128],
                                     lhsT=bt[:, vob:vob + 128],
                                     rhs=bt[:, wyob:wyob + 128],
                                     start=True, stop=True)
                    nc.tensor.matmul(at[:, k * 256 + 128:(k + 1) * 256],
                                     lhsT=bt[:, vob + 128:vob + 256],
                                     rhs=bt[:, wyob:wyob + 128],
                                     start=True, stop=True)
                ats = pats.tile([128, 512], f16, tag="ats", name=f"ats{r}_{p}")
                cp_inst = nc.scalar.copy(ats[:], at[:])
                if p == 0 and tau_insts:
                    # previous round's tau (gates its scan) goes first on ACT
                    add_dep_helper(cp_inst.ins, tau_insts[-1].ins,
                                   reason="pipeline: copies after prev tau")
                for k in range(2):
                    s = p * 2 + k
                    vo, wyo, wxo = _pair_offsets(s)
                    wxs = slice(wxo - p * 1024, wxo - p * 1024 + 128)
                    nc.tensor.matmul(pcf[:, s * 128:(s + 1) * 128],
                                     lhsT=bt[:, wxs],
                                     rhs=ats[:, k * 256:k * 256 + 128],
                                     start=True, stop=True)
                    nc.tensor.matmul(pcd[:, s * 128:(s + 1) * 128],
                                     lhsT=bt[:, wxs],
                                     rhs=ats[:, k * 256 + 128:(k + 1) * 256],
                                     start=True, stop=True)
                # per-pair tau: 1 - S_dens for slices 2p, 2p+1


            pd_v = (pcd[:].rearrange("p (j h) -> p j h", j=B)
                    .rearrange("p j h -> p h j"))
            t_inst = nc.scalar.activation(tau3[:, :, 1:B + 1], pd_v, Ident,
                                          bias=1.0, scale=-1.0)
            tau_insts.append(t_inst)

            if r == n_rounds - 1:
                # host handles the last round's emission: evacuate its feat
                # samples (h-major) for the DMA below
                pfs = pfs_p.tile([128, B * 128], f32, tag="pfs", name="pfs_l")
                nc.scalar.activation(
                    pfs[:].rearrange("p (h j) -> p h j", j=B),
                    pcf[:].rearrange("p (j h) -> p j h", j=B).rearrange(
                        "p j h -> p h j"),
                    Ident)

            s_inst = nc.vector.tensor_tensor_scan(cum[:], tau[:], rcon[:], 1.0,
                                                  mult, add)
            scan_insts.append(s_inst)
            cum3r = cum[:].rearrange("p (h c) -> p h c", h=128)
            if r == 0:
                nc.vector.tensor_copy(carry, cum3r[:, :, B:B + 1])
            elif r == n_rounds - 2:
                nc.vector.tensor_copy(a2, cum3r[:, :, B:B + 1])
            elif r < n_rounds - 1:
                ar = psm.tile([128, 128], f32, tag="ar", name=f"ar{r}")
                nc.vector.tensor_copy(ar[:], cum3r[:, :, B:B + 1])
                a_tiles[r] = ar

            if r >= 1:
                emission(r - 1)

        # last round's emission happens on the host: ship the scan output
        # and the feat samples directly
        nc.sync.dma_start(pfs_d.ap(), pfs[:])
        nc.sync.dma_start(cum_d.ap(), cums[(n_rounds - 1) % 2][:])
        nc.sync.dma_start(outs_d.ap(), outs[:])

    nc.compile()
    _BUILD_CACHE[key] = nc
    return nc


# ------------------------------------------------------------------- driver

def _prepare(image3d, opacity, R, T):
    """Host prep: geometry, active-slice selection, per-core input packing."""
    vol_f = np.asarray(image3d, np.float32).reshape(D, H, W)
    vol_d = (np.asarray(opacity, np.float32) * SCALING).reshape(D, H, W)

    slices = _geometry(R, T)
    active = [p for p, sl in enumerate(slices) if sl is not None]
    # active depth steps are contiguous; shard contiguously so the EA scan
    # splits into per-core segments
    assert active == list(range(active[0], active[-1] + 1))
    n_active = len(active)
    per_core = -(-n_active // N_CORES)
    per_core = -(-per_core // B) * B  # round up to round multiple
    n_rounds = per_core // B

    in_maps = []
    for k in range(N_CORES):
        bl = np.zeros((n_rounds, 128, BLOB_COLS), np.float16)
        for local in range(per_core):
            idx = k * per_core + local
            if idx >= n_active:
                continue  # zero-weight padding slice
            sl = slices[active[idx]]
            r, s = divmod(local, B)
            vo, wyo, wxo = _pair_offsets(s)
            Wy = _axis_weight_matrix(sl["vy"])
            Wx = _axis_weight_matrix(sl["ux"])
            vlerp_f = sl["w0"] * vol_f[sl["z0"]] + sl["w1"] * vol_f[sl["z1"]]
            vlerp_d = sl["w0"] * vol_d[sl["z0"]] + sl["w1"] * vol_d[sl["z1"]]
            bl[r, :, vo:vo + 128] = vlerp_f.astype(np.float16)
            bl[r, :, vo + 128:vo + 256] = vlerp_d.astype(np.float16)
            bl[r, :, wyo:wyo + 128] = Wy.astype(np.float16)
            bl[r, :, wxo:wxo + 128] = Wx.astype(np.float16)
        in_maps.append(
            {"blob": np.ascontiguousarray(
                bl.reshape(n_rounds, 128, B // 2, 1024).transpose(0, 2, 1, 3)
                .reshape(n_rounds * (B // 2), 128, 1024))})
    return in_maps, per_core


def _combine(results):
    """out = sum_k acc_k * prod_{j<k} pk_j, then standardize+normalize."""
    out = np.zeros((128, 128), np.float32)
    trans = np.ones((128, 128), np.float32)
    for r in results:
        o = r["outs"]
        acc0, carry0 = o[:, 0:128], o[:, 128:256]
        e2, a2 = o[:, 256:384], o[:, 384:512]
        cum = r["cum_last"].reshape(128, 128, B + 1)
        pf = r["pfs_last"].reshape(128, 128, B)
        w = cum[:, :, 0:B] - cum[:, :, 1:B + 1]
        e_last = (pf * w).sum(axis=2, dtype=np.float32).astype(np.float32)
        a_last = cum[:, :, B]
        carry1 = carry0 * a2
        acc_k = acc0 + carry0 * e2 + carry1 * e_last
        pk_k = carry1 * a_last
        out = out + trans * acc_k
        trans = trans * pk_k
    g = out[None, None]  # [1,1,W,H] (acc layout is [pixel-x, pixel-y])
    st = (g - g.mean()) / (g.std(ddof=1) + np.float32(1e-8))
    st = (st - st.min() + np.float32(1e-8)) / (st.max() - st.min() + np.float32(1e-8))
    return st.astype(np.float32)


def run(image3d, opacity, R, T, trace=False):
    from concourse.bass_utils import run_bass_kernel_spmd

    in_maps, per_core = _prepare(image3d, opacity, R, T)
    nc = _build_nc(per_core)
    last_exc = None
    for attempt in range(3):
        try:
            res = run_bass_kernel_spmd(nc, in_maps,
                                       core_ids=list(range(N_CORES)),
                                       trace=trace)
            return _combine(res.results), res
        except Exception as e:  # transient NRT device errors: retry
            last_exc = e
            import time as _time
            _time.sleep(2.0)
    raise last_exc


def kernel(image3d, opacity, R, T):
    out, _ = run(image3d, opacity, R, T)
    return out
